# revision 2
# baseline (speedup 1.0000x reference)
"""Trainium2 Bass kernel for local_attention_scalarAdd.

Reference math (per point n of B*H*N points, K=32 neighbors, D=32 dims):
    energy = tanh(q + k^T)            # (K, D)
    scores = energy @ p_add           # (K,)
    attn   = softmax(scores)          # (K,)
    out    = attn @ v                 # (D,)

Layout: points on SBUF partitions, each point's k/v row (K*D=1024 f32 =
4KiB) fully contiguous in DRAM -> perfect DMA. Softmax is computed
without max-subtraction (scores are bounded by sum |tanh|*|p| <= 32 for
the ones-p_add case, exp stays in fp32 range) and normalization is
applied via a reciprocal of the exp-sum.

Engine split (per 512-point chunk, to stay under the DMA-bound pace):
  GPSIMD: energy32 = k + q (f32)
  ACT:    en = tanh(energy32) cast to bf16; exp(scores)
  DVE:    dense tree-reductions in bf16 (2x mode) for both the
          score reduce (over d) and the output reduce (over c), the
          softmax small ops, and w = v*attn (cast to bf16).
The old version used strided tensor_reduce on DVE (1x + strided-AP
penalty) which made DVE the bottleneck at ~340us busy; the tree forms
run dense step-1 so the bf16 levels hit the 2x_1P perf mode.
"""

import sys

sys.path.insert(0, "/opt/trn_rl_repo")

import numpy as np

B, H, N, K, D = 2, 8, 4096, 32, 32
E = K * D  # 1024 elements per point in k/v
P = 128  # SBUF partitions
SUB = 4  # point-groups of 128 per tile -> 512 points/tile
TILE_PTS = P * SUB
N_CORES = 8
PTS_PER_CORE = B * H * N // N_CORES  # 8192
NT = PTS_PER_CORE // TILE_PTS  # 16 tiles per core

_cache = {}


def _build(general_padd: bool):
    import concourse.bacc as bacc
    import concourse.mybir as mybir
    from concourse.tile import TileContext

    f32 = mybir.dt.float32
    bf16 = mybir.dt.bfloat16
    Alu = mybir.AluOpType
    Act = mybir.ActivationFunctionType
    Axis = mybir.AxisListType

    nc = bacc.Bacc("TRN2", target_bir_lowering=False)
    qs = nc.dram_tensor("qs", [PTS_PER_CORE, D], f32, kind="ExternalInput")
    ks = nc.dram_tensor("ks", [PTS_PER_CORE, E], f32, kind="ExternalInput")
    vs = nc.dram_tensor("vs", [PTS_PER_CORE, E], f32, kind="ExternalInput")
    if general_padd:
        pexp = nc.dram_tensor("pexp", [P, D], f32, kind="ExternalInput")
    out = nc.dram_tensor("out", [PTS_PER_CORE, D], f32, kind="ExternalOutput")

    # Ramped segment schedule (in SUB units of 128 points): small tiles at
    # the start so the pipeline fills fast, small at the end so it drains
    # fast. Sums to NT*SUB sub-units.
    total_su = NT * SUB
    if total_su >= 12:
        mid = total_su - 8
        SEGMENTS = (
            [1, 1, 2]
            + [4] * (mid // 4)
            + ([mid % 4] if mid % 4 else [])
            + [2, 1, 1]
        )
    else:
        SEGMENTS = []
        rem = total_su
        while rem:
            s = min(4, rem)
            SEGMENTS.append(s)
            rem -= s
    assert sum(SEGMENTS) == total_su

    ks_s = ks[:].rearrange("(s p) e -> p s e", p=P)
    vs_s = vs[:].rearrange("(s p) e -> p s e", p=P)
    out_s = out[:].rearrange("(s p) d -> p s d", p=P)

    with TileContext(nc) as tc:
        with (
            tc.tile_pool(name="big", bufs=3) as big,
            tc.tile_pool(name="en32p", bufs=3) as en32p,
            tc.tile_pool(name="enbfp", bufs=3) as enbfp,
            tc.tile_pool(name="small", bufs=3) as small,
            tc.tile_pool(name="const", bufs=1) as const,
        ):
            if general_padd:
                p_t = const.tile([P, D], f32, tag="padd")
                nc.sync.dma_start(out=p_t[:], in_=pexp[:])

            # q for the whole shard: a small first piece so the first adds
            # are unblocked quickly, then the rest.
            q_full = const.tile([P, NT * SUB * D], f32, tag="q")
            QSPLIT = min(8, NT * SUB)
            nc.sync.dma_start(
                out=q_full[:, : QSPLIT * D].rearrange("p (s d) -> p s d", d=D),
                in_=qs[:].rearrange("(s p) d -> p s d", p=P)[:, :QSPLIT],
            )
            su0 = 0  # sub-unit cursor
            for seg_idx, nsub in enumerate(SEGMENTS):
                if seg_idx == 3 and NT * SUB > QSPLIT:
                    # bulk of q arrives after the pipeline is rolling
                    nc.sync.dma_start(
                        out=q_full[:, QSPLIT * D :].rearrange(
                            "p (s d) -> p s d", d=D
                        ),
                        in_=qs[:].rearrange("(s p) d -> p s d", p=P)[
                            :, QSPLIT:
                        ],
                    )
                k_seg = big.tile([P, SUB * E], f32, tag="k")
                v_seg = big.tile([P, SUB * E], f32, tag="v")
                nc.sync.dma_start(
                    out=k_seg[:, : nsub * E].rearrange(
                        "p (s e) -> p s e", s=nsub
                    ),
                    in_=ks_s[:, su0 : su0 + nsub],
                )
                # v rides the second HWDGE queue (ACT) so k and v transfers
                # overlap instead of serializing on one queue.
                nc.scalar.dma_start(
                    out=v_seg[:, : nsub * E].rearrange(
                        "p (s e) -> p s e", s=nsub
                    ),
                    in_=vs_s[:, su0 : su0 + nsub],
                )

                done = 0
                while done < nsub:
                    cs = min(SUB, nsub - done)  # chunk size in sub-units
                    su = su0 + done
                    k_t = k_seg[:, done * E : (done + cs) * E]
                    v_t = v_seg[:, done * E : (done + cs) * E]
                    done += cs

                    en32 = en32p.tile([P, SUB * E], f32, tag="en32")
                    # enbf holds tanh energies, then is reused as the
                    # attn-weighted-v (w) buffer once the score tree has
                    # consumed the energies.
                    enbf = enbfp.tile([P, SUB * E], bf16, tag="enbf")
                    sc3 = small.tile([P, SUB * (E // 8)], f32, tag="sc3")
                    sc = small.tile([P, SUB * K], f32, tag="sc")
                    ex = small.tile([P, SUB * K], f32, tag="ex")
                    rs = small.tile([P, SUB], f32, tag="rs")
                    ri = small.tile([P, SUB], f32, tag="ri")
                    at = small.tile([P, SUB * K], f32, tag="at")
                    on = small.tile([P, SUB * D], f32, tag="on")
                    q_t = q_full[:, su * D : (su + cs) * D]

                    # energy = k + q (q broadcast over the K axis). k layout
                    # per sub-block is d-major: elem = d*K + c, rows r = t*D+d
                    # are contiguous 32-elem runs, q contiguous over r.
                    # TT instructions support at most 2 free AP dims.
                    k_ap = k_t.rearrange("p (r c) -> p r c", c=K)
                    en32_ap = en32[:, : cs * E].rearrange(
                        "p (r c) -> p r c", c=K
                    )
                    q_ap = q_t.unsqueeze(2).broadcast_to([P, cs * D, K])
                    # GPSIMD does the add: DVE is the bottleneck and Pool is
                    # otherwise idle.
                    nc.gpsimd.tensor_tensor(en32_ap, k_ap, q_ap, Alu.add)

                    # tanh with downcast to bf16 (ACT is 1 elem/cyc/lane
                    # regardless of dtype; the cast is free here)
                    nc.scalar.activation(
                        enbf[:, : cs * E], en32[:, : cs * E], Act.Tanh
                    )

                    if general_padd:
                        pb = p_t[:].unsqueeze(2).broadcast_to([P, D, K])
                        for t in range(cs):
                            sl = enbf[:, t * E : (t + 1) * E].rearrange(
                                "p (d c) -> p d c", c=K
                            )
                            nc.vector.tensor_tensor(sl, sl, pb, Alu.mult)

                    # scores[t,c] = sum_d en[t,d,c] via a dense tree over d
                    # (d-major layout: the top/bottom d-halves are contiguous
                    # 512-elem runs). Levels 1-2 stay bf16 (2x DVE mode),
                    # levels 3-5 accumulate in f32 for precision.
                    hE = E // 2  # 512
                    el1 = enbf[:, : cs * E].rearrange(
                        "p (t e) -> p t e", t=cs
                    )
                    nc.vector.tensor_tensor(
                        el1[:, :, :hE], el1[:, :, :hE], el1[:, :, hE:], Alu.add
                    )
                    nc.vector.tensor_tensor(
                        el1[:, :, : hE // 2],
                        el1[:, :, : hE // 2],
                        el1[:, :, hE // 2 : hE],
                        Alu.add,
                    )
                    qE = E // 4  # 256
                    sc3_ap = sc3[:, : cs * (E // 8)].rearrange(
                        "p (t e) -> p t e", t=cs
                    )
                    nc.vector.tensor_tensor(
                        sc3_ap,
                        el1[:, :, : qE // 2],
                        el1[:, :, qE // 2 : qE],
                        Alu.add,
                    )
                    nc.vector.tensor_tensor(
                        sc3_ap[:, :, : E // 16],
                        sc3_ap[:, :, : E // 16],
                        sc3_ap[:, :, E // 16 : E // 8],
                        Alu.add,
                    )
                    sc_ap = sc[:, : cs * K].rearrange("p (t c) -> p t c", t=cs)
                    nc.vector.tensor_tensor(
                        sc_ap,
                        sc3_ap[:, :, :K],
                        sc3_ap[:, :, K : 2 * K],
                        Alu.add,
                    )

                    # softmax over the K axis (no max subtraction needed:
                    # |scores| <= 32 so exp stays in fp32 range)
                    nc.scalar.activation(
                        ex[:, : cs * K], sc[:, : cs * K], Act.Exp
                    )
                    nc.vector.tensor_reduce(
                        rs[:, :cs],
                        ex[:, : cs * K].rearrange("p (t c) -> p t c", t=cs),
                        axis=Axis.X,
                        op=Alu.add,
                    )
                    nc.vector.reciprocal(ri[:, :cs], rs[:, :cs])
                    ri_b = ri[:, :cs].unsqueeze(2).broadcast_to([P, cs, K])
                    at_ap = at[:, : cs * K].rearrange("p (t c) -> p t c", t=cs)
                    nc.vector.tensor_tensor(
                        at_ap,
                        ex[:, : cs * K].rearrange("p (t c) -> p t c", t=cs),
                        ri_b,
                        Alu.mult,
                    )

                    # w[t,c,d] = v[t,c,d] * attn[t,c] (rows s = t*K + c),
                    # written into the enbf buffer as bf16 (energy is
                    # consumed by the score tree).
                    v_ap = v_t.rearrange("p (s d) -> p s d", d=D)
                    w_ap = enbf[:, : cs * E].rearrange("p (s d) -> p s d", d=D)
                    at_b = at[:, : cs * K].unsqueeze(2).broadcast_to(
                        [P, cs * K, D]
                    )
                    nc.vector.tensor_tensor(w_ap, v_ap, at_b, Alu.mult)

                    # out[t,d] = sum_c w[t,c,d] via a dense tree over c
                    # (c-major layout). Levels 1-4 bf16 (2x), level 5 f32.
                    wl = enbf[:, : cs * E].rearrange("p (t e) -> p t e", t=cs)
                    nc.vector.tensor_tensor(
                        wl[:, :, :hE], wl[:, :, :hE], wl[:, :, hE:], Alu.add
                    )
                    nc.vector.tensor_tensor(
                        wl[:, :, : hE // 2],
                        wl[:, :, : hE // 2],
                        wl[:, :, hE // 2 : hE],
                        Alu.add,
                    )
                    nc.vector.tensor_tensor(
                        wl[:, :, : qE // 2],
                        wl[:, :, : qE // 2],
                        wl[:, :, qE // 2 : qE],
                        Alu.add,
                    )
                    nc.vector.tensor_tensor(
                        wl[:, :, : E // 16],
                        wl[:, :, : E // 16],
                        wl[:, :, E // 16 : E // 8],
                        Alu.add,
                    )
                    on_ap = on[:, : cs * D].rearrange("p (t d) -> p t d", t=cs)
                    nc.vector.tensor_tensor(
                        on_ap,
                        wl[:, :, :D],
                        wl[:, :, D : 2 * D],
                        Alu.add,
                    )

                    nc.sync.dma_start(
                        out=out_s[:, su : su + cs],
                        in_=on[:, : cs * D].rearrange("p (s d) -> p s d", s=cs),
                    )
                su0 += nsub
            if len(SEGMENTS) <= 3 and NT * SUB > QSPLIT:
                nc.sync.dma_start(
                    out=q_full[:, QSPLIT * D :].rearrange(
                        "p (s d) -> p s d", d=D
                    ),
                    in_=qs[:].rearrange("(s p) d -> p s d", p=P)[:, QSPLIT:],
                )

    return nc


def _get_nc(general_padd: bool):
    key = bool(general_padd)
    if key not in _cache:
        nc = _build(general_padd)
        # Run the Bacc compile pipeline (register allocation, sync-wait
        # splitting, ACT table loads) before handing the module to the
        # PJRT execution path, which serializes nc.m as-is.
        nc.finalize()
        _cache[key] = nc
    return _cache[key]


def _shard(q, k, v, p_add):
    """Returns in_maps for the 8 cores. Core c gets flattened-(B*H) groups
    [2c, 2c+1]."""
    qf = np.ascontiguousarray(q, dtype=np.float32).reshape(B * H, N, D)
    kf = np.ascontiguousarray(k, dtype=np.float32).reshape(B * H, N, E)
    vf = np.ascontiguousarray(v, dtype=np.float32).reshape(B * H, N, E)
    gpc = B * H // N_CORES  # bh-groups per core (2)
    general = not np.allclose(np.asarray(p_add, dtype=np.float32), 1.0)
    in_maps = []
    for c in range(N_CORES):
        m = {
            "qs": np.ascontiguousarray(
                qf[c * gpc : (c + 1) * gpc].reshape(PTS_PER_CORE, D)
            ),
            "ks": np.ascontiguousarray(
                kf[c * gpc : (c + 1) * gpc].reshape(PTS_PER_CORE, E)
            ),
            "vs": np.ascontiguousarray(
                vf[c * gpc : (c + 1) * gpc].reshape(PTS_PER_CORE, E)
            ),
        }
        if general:
            m["pexp"] = np.ascontiguousarray(
                np.tile(
                    np.asarray(p_add, dtype=np.float32).reshape(1, D), (P, 1)
                )
            )
        in_maps.append(m)
    return in_maps, general


def _run(q, k, v, p_add, trace=False, tmpdir=None):
    from concourse.bass_utils import run_bass_kernel_spmd

    in_maps, general = _shard(q, k, v, p_add)
    nc = _get_nc(general)
    res = run_bass_kernel_spmd(
        nc, in_maps, list(range(N_CORES)), trace=trace, tmpdir=tmpdir
    )
    gpc = B * H // N_CORES
    out_full = np.empty((B, N, H, D), dtype=np.float32)
    for c in range(N_CORES):
        o = res.results[c]["out"].reshape(gpc, N, D)
        for j in range(gpc):
            bh = c * gpc + j
            out_full[bh // H, :, bh % H, :] = o[j]
    return out_full, res


def kernel(q, k, v, p_add):
    out, _ = _run(q, k, v, p_add)
    return out


# revision 7
# speedup vs baseline: 1.0701x; 1.0701x over previous
"""Trainium2 Bass kernel for local_attention_scalarAdd.

Reference math (per point n of B*H*N points, K=32 neighbors, D=32 dims):
    energy = tanh(q + k^T)            # (K, D)
    scores = energy @ p_add           # (K,)
    attn   = softmax(scores)          # (K,)
    out    = attn @ v                 # (D,)

Layout: points on SBUF partitions, each point's k/v row (K*D=1024 f32 =
4KiB) fully contiguous in DRAM -> perfect DMA. Softmax is computed
without max-subtraction (scores are bounded by sum |tanh|*|p| <= 32 for
the ones-p_add case, exp stays in fp32 range) and normalization is
applied via a reciprocal of the exp-sum.

Engine split (per 512-point chunk, to stay under the DMA-bound pace):
  GPSIMD: energy32 = k + q (f32); v arrives via SWDGE DMA with an
          inline f32->bf16 cast (free: the HBM read side is the limit)
  ACT:    en = tanh(energy32) cast to bf16; exp(scores); attn expanded
          to a dense bf16 [.., K, D] replica (at_rep)
  DVE:    dense tree-reductions in bf16 (2x mode) for both the
          score reduce (over d) and the output reduce (over c), the
          softmax small ops, and w = v_bf16*at_rep (dense bf16 2x).
The old version used strided tensor_reduce on DVE (1x + strided-AP
penalty) which made DVE the bottleneck at ~340us busy; the tree forms
run dense step-1 so the bf16 levels hit the 2x_1P perf mode.
"""

import sys

sys.path.insert(0, "/opt/trn_rl_repo")

import numpy as np

B, H, N, K, D = 2, 8, 4096, 32, 32
E = K * D  # 1024 elements per point in k/v
P = 128  # SBUF partitions
SUB = 4  # point-groups of 128 per tile -> 512 points/tile
TILE_PTS = P * SUB
N_CORES = 8
PTS_PER_CORE = B * H * N // N_CORES  # 8192
NT = PTS_PER_CORE // TILE_PTS  # 16 tiles per core

_cache = {}


def _build(general_padd: bool):
    import concourse.bacc as bacc
    import concourse.mybir as mybir
    from concourse.tile import TileContext

    f32 = mybir.dt.float32
    bf16 = mybir.dt.bfloat16
    Alu = mybir.AluOpType
    Act = mybir.ActivationFunctionType
    Axis = mybir.AxisListType

    nc = bacc.Bacc("TRN2", target_bir_lowering=False)
    qs = nc.dram_tensor("qs", [PTS_PER_CORE, D], f32, kind="ExternalInput")
    ks = nc.dram_tensor("ks", [PTS_PER_CORE, E], f32, kind="ExternalInput")
    vs = nc.dram_tensor("vs", [PTS_PER_CORE, E], f32, kind="ExternalInput")
    if general_padd:
        pexp = nc.dram_tensor("pexp", [P, D], f32, kind="ExternalInput")
    out = nc.dram_tensor("out", [PTS_PER_CORE, D], f32, kind="ExternalOutput")

    # Ramped segment schedule (in SUB units of 128 points): small tiles at
    # the start so the pipeline fills fast, small at the end so it drains
    # fast. Sums to NT*SUB sub-units.
    total_su = NT * SUB
    if total_su >= 12:
        mid = total_su - 8
        SEGMENTS = (
            [1, 1, 2]
            + [4] * (mid // 4)
            + ([mid % 4] if mid % 4 else [])
            + [2, 1, 1]
        )
    else:
        SEGMENTS = []
        rem = total_su
        while rem:
            s = min(4, rem)
            SEGMENTS.append(s)
            rem -= s
    assert sum(SEGMENTS) == total_su

    ks_s = ks[:].rearrange("(s p) e -> p s e", p=P)
    vs_s = vs[:].rearrange("(s p) e -> p s e", p=P)
    out_s = out[:].rearrange("(s p) d -> p s d", p=P)

    with TileContext(nc) as tc:
        with (
            tc.tile_pool(name="big", bufs=3) as big,
            tc.tile_pool(name="en32p", bufs=3) as en32p,
            tc.tile_pool(name="enbfp", bufs=3) as enbfp,
            tc.tile_pool(name="small", bufs=3) as small,
            tc.tile_pool(name="const", bufs=1) as const,
        ):
            if general_padd:
                # bf16 so the energy multiply matches enbf's dtype
                p_t = const.tile([P, D], bf16, tag="padd")
                nc.gpsimd.dma_start(out=p_t[:], in_=pexp[:])

            # q for the whole shard: a small first piece so the first adds
            # are unblocked quickly, then the rest.
            q_full = const.tile([P, NT * SUB * D], f32, tag="q")
            QSPLIT = min(8, NT * SUB)
            nc.sync.dma_start(
                out=q_full[:, : QSPLIT * D].rearrange("p (s d) -> p s d", d=D),
                in_=qs[:].rearrange("(s p) d -> p s d", p=P)[:, :QSPLIT],
            )
            su0 = 0  # sub-unit cursor
            for seg_idx, nsub in enumerate(SEGMENTS):
                if seg_idx == 3 and NT * SUB > QSPLIT:
                    # bulk of q arrives after the pipeline is rolling
                    nc.sync.dma_start(
                        out=q_full[:, QSPLIT * D :].rearrange(
                            "p (s d) -> p s d", d=D
                        ),
                        in_=qs[:].rearrange("(s p) d -> p s d", p=P)[
                            :, QSPLIT:
                        ],
                    )
                k_seg = big.tile([P, SUB * E], f32, tag="k")
                v_seg = big.tile([P, SUB * E], bf16, tag="v")
                nc.sync.dma_start(
                    out=k_seg[:, : nsub * E].rearrange(
                        "p (s e) -> p s e", s=nsub
                    ),
                    in_=ks_s[:, su0 : su0 + nsub],
                )
                # v rides the SWDGE queue with an inline f32->bf16 cast;
                # the HBM read side (f32) is the bandwidth limit either
                # way, and bf16 v lets the w-multiply run in the DVE 2x
                # perf mode.
                nc.gpsimd.dma_start(
                    out=v_seg[:, : nsub * E].rearrange(
                        "p (s e) -> p s e", s=nsub
                    ),
                    in_=vs_s[:, su0 : su0 + nsub],
                )

                done = 0
                while done < nsub:
                    cs = min(SUB, nsub - done)  # chunk size in sub-units
                    su = su0 + done
                    k_t = k_seg[:, done * E : (done + cs) * E]
                    v_t = v_seg[:, done * E : (done + cs) * E]
                    done += cs

                    en32 = en32p.tile([P, SUB * E], f32, tag="en32")
                    # enbf holds tanh energies, then is reused as the
                    # attn-weighted-v (w) buffer once the score tree has
                    # consumed the energies.
                    enbf = enbfp.tile([P, SUB * E], bf16, tag="enbf")
                    sc3 = small.tile([P, SUB * (E // 8)], f32, tag="sc3")
                    sc = small.tile([P, SUB * K], f32, tag="sc")
                    ex = small.tile([P, SUB * K], f32, tag="ex")
                    rs = small.tile([P, SUB], f32, tag="rs")
                    ri = small.tile([P, SUB], f32, tag="ri")
                    at = small.tile([P, SUB * K], f32, tag="at")
                    on = small.tile([P, SUB * D], f32, tag="on")
                    q_t = q_full[:, su * D : (su + cs) * D]

                    # energy = k + q (q broadcast over the K axis). k layout
                    # per sub-block is d-major: elem = d*K + c, rows r = t*D+d
                    # are contiguous 32-elem runs, q contiguous over r.
                    # TT instructions support at most 2 free AP dims.
                    k_ap = k_t.rearrange("p (r c) -> p r c", c=K)
                    en32_ap = en32[:, : cs * E].rearrange(
                        "p (r c) -> p r c", c=K
                    )
                    q_ap = q_t.unsqueeze(2).broadcast_to([P, cs * D, K])
                    # GPSIMD does the add: DVE is the bottleneck and Pool is
                    # otherwise idle.
                    nc.gpsimd.tensor_tensor(en32_ap, k_ap, q_ap, Alu.add)

                    # tanh with downcast to bf16 (ACT is 1 elem/cyc/lane
                    # regardless of dtype; the cast is free here)
                    nc.scalar.activation(
                        enbf[:, : cs * E], en32[:, : cs * E], Act.Tanh
                    )

                    if general_padd:
                        pb = p_t[:].unsqueeze(2).broadcast_to([P, D, K])
                        for t in range(cs):
                            sl = enbf[:, t * E : (t + 1) * E].rearrange(
                                "p (d c) -> p d c", c=K
                            )
                            nc.vector.tensor_tensor(sl, sl, pb, Alu.mult)

                    # scores[t,c] = sum_d en[t,d,c] via a dense tree over d
                    # (d-major layout: the top/bottom d-halves are contiguous
                    # 512-elem runs). Levels 1-2 stay bf16 (2x DVE mode),
                    # levels 3-5 accumulate in f32 for precision.
                    hE = E // 2  # 512
                    el1 = enbf[:, : cs * E].rearrange(
                        "p (t e) -> p t e", t=cs
                    )
                    nc.vector.tensor_tensor(
                        el1[:, :, :hE], el1[:, :, :hE], el1[:, :, hE:], Alu.add
                    )
                    nc.vector.tensor_tensor(
                        el1[:, :, : hE // 2],
                        el1[:, :, : hE // 2],
                        el1[:, :, hE // 2 : hE],
                        Alu.add,
                    )
                    qE = E // 4  # 256
                    sc3_ap = sc3[:, : cs * (E // 8)].rearrange(
                        "p (t e) -> p t e", t=cs
                    )
                    nc.vector.tensor_tensor(
                        sc3_ap,
                        el1[:, :, : qE // 2],
                        el1[:, :, qE // 2 : qE],
                        Alu.add,
                    )
                    nc.vector.tensor_tensor(
                        sc3_ap[:, :, : E // 16],
                        sc3_ap[:, :, : E // 16],
                        sc3_ap[:, :, E // 16 : E // 8],
                        Alu.add,
                    )
                    sc_ap = sc[:, : cs * K].rearrange("p (t c) -> p t c", t=cs)
                    nc.vector.tensor_tensor(
                        sc_ap,
                        sc3_ap[:, :, :K],
                        sc3_ap[:, :, K : 2 * K],
                        Alu.add,
                    )

                    # softmax over the K axis (no max subtraction needed:
                    # |scores| <= 32 so exp stays in fp32 range)
                    nc.scalar.activation(
                        ex[:, : cs * K], sc[:, : cs * K], Act.Exp
                    )
                    nc.vector.tensor_reduce(
                        rs[:, :cs],
                        ex[:, : cs * K].rearrange("p (t c) -> p t c", t=cs),
                        axis=Axis.X,
                        op=Alu.add,
                    )
                    nc.vector.reciprocal(ri[:, :cs], rs[:, :cs])
                    ri_b = ri[:, :cs].unsqueeze(2).broadcast_to([P, cs, K])
                    at_ap = at[:, : cs * K].rearrange("p (t c) -> p t c", t=cs)
                    nc.vector.tensor_tensor(
                        at_ap,
                        ex[:, : cs * K].rearrange("p (t c) -> p t c", t=cs),
                        ri_b,
                        Alu.mult,
                    )

                    # attn expanded to a dense bf16 replica (at_rep[s,d] =
                    # attn[s] for s = t*K+c) on ACT, into the en32 buffer
                    # (bitcast to bf16; the energies there are consumed).
                    at_rep = en32[:, : cs * E // 2].bitcast(bf16)
                    at_b = at[:, : cs * K].unsqueeze(2).broadcast_to(
                        [P, cs * K, D]
                    )
                    nc.scalar.activation(
                        at_rep.rearrange("p (s d) -> p s d", d=D),
                        at_b,
                        Act.Copy,
                    )

                    # w[t,c,d] = v[t,c,d] * attn[t,c]: dense bf16 * bf16 ->
                    # bf16 (DVE 2x mode), written into the enbf buffer.
                    nc.vector.tensor_tensor(
                        enbf[:, : cs * E], v_t, at_rep, Alu.mult
                    )

                    # out[t,d] = sum_c w[t,c,d] via a dense tree over c
                    # (c-major layout). Levels 1-4 bf16 (2x), level 5 f32.
                    wl = enbf[:, : cs * E].rearrange("p (t e) -> p t e", t=cs)
                    nc.vector.tensor_tensor(
                        wl[:, :, :hE], wl[:, :, :hE], wl[:, :, hE:], Alu.add
                    )
                    nc.vector.tensor_tensor(
                        wl[:, :, : hE // 2],
                        wl[:, :, : hE // 2],
                        wl[:, :, hE // 2 : hE],
                        Alu.add,
                    )
                    nc.vector.tensor_tensor(
                        wl[:, :, : qE // 2],
                        wl[:, :, : qE // 2],
                        wl[:, :, qE // 2 : qE],
                        Alu.add,
                    )
                    nc.vector.tensor_tensor(
                        wl[:, :, : E // 16],
                        wl[:, :, : E // 16],
                        wl[:, :, E // 16 : E // 8],
                        Alu.add,
                    )
                    on_ap = on[:, : cs * D].rearrange("p (t d) -> p t d", t=cs)
                    nc.vector.tensor_tensor(
                        on_ap,
                        wl[:, :, :D],
                        wl[:, :, D : 2 * D],
                        Alu.add,
                    )

                    nc.sync.dma_start(
                        out=out_s[:, su : su + cs],
                        in_=on[:, : cs * D].rearrange("p (s d) -> p s d", s=cs),
                    )
                su0 += nsub
            if len(SEGMENTS) <= 3 and NT * SUB > QSPLIT:
                nc.sync.dma_start(
                    out=q_full[:, QSPLIT * D :].rearrange(
                        "p (s d) -> p s d", d=D
                    ),
                    in_=qs[:].rearrange("(s p) d -> p s d", p=P)[:, QSPLIT:],
                )

    return nc


def _get_nc(general_padd: bool):
    key = bool(general_padd)
    if key not in _cache:
        nc = _build(general_padd)
        # Run the Bacc compile pipeline (register allocation, sync-wait
        # splitting, ACT table loads) before handing the module to the
        # PJRT execution path, which serializes nc.m as-is.
        nc.finalize()
        _cache[key] = nc
    return _cache[key]


def _shard(q, k, v, p_add):
    """Returns in_maps for the 8 cores. Core c gets flattened-(B*H) groups
    [2c, 2c+1]."""
    qf = np.ascontiguousarray(q, dtype=np.float32).reshape(B * H, N, D)
    kf = np.ascontiguousarray(k, dtype=np.float32).reshape(B * H, N, E)
    vf = np.ascontiguousarray(v, dtype=np.float32).reshape(B * H, N, E)
    gpc = B * H // N_CORES  # bh-groups per core (2)
    general = not np.allclose(np.asarray(p_add, dtype=np.float32), 1.0)
    in_maps = []
    for c in range(N_CORES):
        m = {
            "qs": np.ascontiguousarray(
                qf[c * gpc : (c + 1) * gpc].reshape(PTS_PER_CORE, D)
            ),
            "ks": np.ascontiguousarray(
                kf[c * gpc : (c + 1) * gpc].reshape(PTS_PER_CORE, E)
            ),
            "vs": np.ascontiguousarray(
                vf[c * gpc : (c + 1) * gpc].reshape(PTS_PER_CORE, E)
            ),
        }
        if general:
            m["pexp"] = np.ascontiguousarray(
                np.tile(
                    np.asarray(p_add, dtype=np.float32).reshape(1, D), (P, 1)
                )
            )
        in_maps.append(m)
    return in_maps, general


def _run(q, k, v, p_add, trace=False, tmpdir=None):
    from concourse.bass_utils import run_bass_kernel_spmd

    in_maps, general = _shard(q, k, v, p_add)
    nc = _get_nc(general)
    res = run_bass_kernel_spmd(
        nc, in_maps, list(range(N_CORES)), trace=trace, tmpdir=tmpdir
    )
    gpc = B * H // N_CORES
    out_full = np.empty((B, N, H, D), dtype=np.float32)
    for c in range(N_CORES):
        o = res.results[c]["out"].reshape(gpc, N, D)
        for j in range(gpc):
            bh = c * gpc + j
            out_full[bh // H, :, bh % H, :] = o[j]
    return out_full, res


def kernel(q, k, v, p_add):
    out, _ = _run(q, k, v, p_add)
    return out


# revision 12
# speedup vs baseline: 1.3377x; 1.2500x over previous
"""Trainium2 Bass kernel for local_attention_scalarAdd.

Reference math (per point n of B*H*N points, K=32 neighbors, D=32 dims):
    energy = tanh(q + k^T)            # (K, D)
    scores = energy @ p_add           # (K,)
    attn   = softmax(scores)          # (K,)
    out    = attn @ v                 # (D,)

Layout: points on SBUF partitions, each point's k/v row (K*D=1024 f32 =
4KiB) fully contiguous in DRAM -> perfect DMA. Softmax is computed
without max-subtraction (scores are bounded by sum |tanh|*|p| <= 32 for
the ones-p_add case, exp stays in fp32 range) and normalization is
applied via a reciprocal of the exp-sum.

Engine split (per 512-point chunk, to stay under the DMA-bound pace):
  DMA:    k and v arrive via SWDGE DMA with an inline f32->bf16 cast
          (free: the HBM read side is the bandwidth limit either way)
  ACT:    q expanded to a dense bf16 [.., D, K] replica (q_rep);
          tanh in place; exp(scores); attn expanded to a dense bf16
          replica (at_rep)
  DVE:    energy = k + q_rep (dense bf16 2x); dense tree-reductions in
          bf16 (2x mode) for the score reduce (over d) and the output
          reduce (over c); softmax small ops; w = v*at_rep (bf16 2x).
  GPSIMD: nothing but SWDGE DMA triggers. GPSIMD compute serializes
          with DVE on the shared SBUF port pair (measured: a 7us
          GPSIMD add blocks a concurrent DVE tensor_tensor for its
          entire duration), so putting compute there buys nothing.
The original version used strided tensor_reduce on DVE (1x + strided-AP
penalty) which made DVE the bottleneck at ~340us busy; the tree forms
run dense step-1 so the bf16 levels hit the 2x_1P perf mode.
"""

import sys

sys.path.insert(0, "/opt/trn_rl_repo")

import numpy as np

B, H, N, K, D = 2, 8, 4096, 32, 32
E = K * D  # 1024 elements per point in k/v
P = 128  # SBUF partitions
SUB = 4  # point-groups of 128 per tile -> 512 points/tile
TILE_PTS = P * SUB
N_CORES = 8
PTS_PER_CORE = B * H * N // N_CORES  # 8192
NT = PTS_PER_CORE // TILE_PTS  # 16 tiles per core

_cache = {}


def _build(general_padd: bool):
    import concourse.bacc as bacc
    import concourse.mybir as mybir
    from concourse.tile import TileContext

    f32 = mybir.dt.float32
    bf16 = mybir.dt.bfloat16
    Alu = mybir.AluOpType
    Act = mybir.ActivationFunctionType
    Axis = mybir.AxisListType

    nc = bacc.Bacc("TRN2", target_bir_lowering=False)
    qs = nc.dram_tensor("qs", [PTS_PER_CORE, D], f32, kind="ExternalInput")
    ks = nc.dram_tensor("ks", [PTS_PER_CORE, E], f32, kind="ExternalInput")
    vs = nc.dram_tensor("vs", [PTS_PER_CORE, E], f32, kind="ExternalInput")
    if general_padd:
        pexp = nc.dram_tensor("pexp", [P, D], f32, kind="ExternalInput")
    out = nc.dram_tensor("out", [PTS_PER_CORE, D], f32, kind="ExternalOutput")

    # Ramped segment schedule (in SUB units of 128 points): small tiles at
    # the start so the pipeline fills fast, small at the end so it drains
    # fast. Sums to NT*SUB sub-units.
    total_su = NT * SUB
    if total_su >= 12:
        mid = total_su - 8
        SEGMENTS = (
            [1, 1, 2]
            + [4] * (mid // 4)
            + ([mid % 4] if mid % 4 else [])
            + [2, 1, 1]
        )
    else:
        SEGMENTS = []
        rem = total_su
        while rem:
            s = min(4, rem)
            SEGMENTS.append(s)
            rem -= s
    assert sum(SEGMENTS) == total_su

    ks_s = ks[:].rearrange("(s p) e -> p s e", p=P)
    vs_s = vs[:].rearrange("(s p) e -> p s e", p=P)
    out_s = out[:].rearrange("(s p) d -> p s d", p=P)

    with TileContext(nc) as tc:
        with (
            tc.tile_pool(name="big", bufs=3) as big,
            tc.tile_pool(name="en32p", bufs=3) as en32p,
            tc.tile_pool(name="enbfp", bufs=3) as enbfp,
            tc.tile_pool(name="small", bufs=3) as small,
            tc.tile_pool(name="const", bufs=1) as const,
        ):
            if general_padd:
                # bf16 so the energy multiply matches enbf's dtype
                p_t = const.tile([P, D], bf16, tag="padd")
                nc.gpsimd.dma_start(out=p_t[:], in_=pexp[:])

            # q for the whole shard: a small first piece so the first adds
            # are unblocked quickly, then the rest.
            q_full = const.tile([P, NT * SUB * D], f32, tag="q")
            QSPLIT = min(8, NT * SUB)
            nc.sync.dma_start(
                out=q_full[:, : QSPLIT * D].rearrange("p (s d) -> p s d", d=D),
                in_=qs[:].rearrange("(s p) d -> p s d", p=P)[:, :QSPLIT],
            )
            su0 = 0  # sub-unit cursor
            for seg_idx, nsub in enumerate(SEGMENTS):
                if seg_idx == 3 and NT * SUB > QSPLIT:
                    # bulk of q arrives after the pipeline is rolling
                    nc.sync.dma_start(
                        out=q_full[:, QSPLIT * D :].rearrange(
                            "p (s d) -> p s d", d=D
                        ),
                        in_=qs[:].rearrange("(s p) d -> p s d", p=P)[
                            :, QSPLIT:
                        ],
                    )
                k_seg = big.tile([P, SUB * E], bf16, tag="k")
                v_seg = big.tile([P, SUB * E], bf16, tag="v")
                # k and v ride the SWDGE queue with an inline f32->bf16
                # cast; the HBM read side (f32) is the bandwidth limit
                # either way, and bf16 operands let the energy add and
                # the w-multiply run in the DVE 2x perf mode.
                nc.gpsimd.dma_start(
                    out=k_seg[:, : nsub * E].rearrange(
                        "p (s e) -> p s e", s=nsub
                    ),
                    in_=ks_s[:, su0 : su0 + nsub],
                )
                nc.gpsimd.dma_start(
                    out=v_seg[:, : nsub * E].rearrange(
                        "p (s e) -> p s e", s=nsub
                    ),
                    in_=vs_s[:, su0 : su0 + nsub],
                )

                done = 0
                while done < nsub:
                    cs = min(SUB, nsub - done)  # chunk size in sub-units
                    su = su0 + done
                    k_t = k_seg[:, done * E : (done + cs) * E]
                    v_t = v_seg[:, done * E : (done + cs) * E]
                    done += cs

                    q_rep = en32p.tile([P, SUB * E], bf16, tag="qrep")
                    at_rep = en32p.tile([P, SUB * E], bf16, tag="atrep")
                    # enbf holds tanh energies, then is reused as the
                    # attn-weighted-v (w) buffer once the score tree has
                    # consumed the energies.
                    enbf = enbfp.tile([P, SUB * E], bf16, tag="enbf")
                    sc3 = small.tile([P, SUB * (E // 8)], f32, tag="sc3")
                    sc = small.tile([P, SUB * K], f32, tag="sc")
                    ex = small.tile([P, SUB * K], f32, tag="ex")
                    rs = small.tile([P, SUB], f32, tag="rs")
                    ri = small.tile([P, SUB], f32, tag="ri")
                    at = small.tile([P, SUB * K], f32, tag="at")
                    on = small.tile([P, SUB * D], f32, tag="on")
                    q_t = q_full[:, su * D : (su + cs) * D]

                    # q expanded to a dense bf16 replica on ACT: q_rep[r,c]
                    # = q[r] for rows r = t*D+d (k layout per sub-block is
                    # d-major: elem = d*K + c).
                    q_ap = q_t.unsqueeze(2).broadcast_to([P, cs * D, K])
                    nc.scalar.activation(
                        q_rep[:, : cs * E].rearrange("p (r c) -> p r c", c=K),
                        q_ap,
                        Act.Copy,
                    )

                    # energy = k + q_rep: dense bf16 + bf16 (DVE 2x mode)
                    nc.vector.tensor_tensor(
                        enbf[:, : cs * E],
                        k_t,
                        q_rep[:, : cs * E],
                        Alu.add,
                    )

                    # tanh in place (ACT is 1 elem/cyc/lane regardless of
                    # dtype)
                    nc.scalar.activation(
                        enbf[:, : cs * E], enbf[:, : cs * E], Act.Tanh
                    )

                    if general_padd:
                        pb = p_t[:].unsqueeze(2).broadcast_to([P, D, K])
                        for t in range(cs):
                            sl = enbf[:, t * E : (t + 1) * E].rearrange(
                                "p (d c) -> p d c", c=K
                            )
                            nc.vector.tensor_tensor(sl, sl, pb, Alu.mult)

                    # scores[t,c] = sum_d en[t,d,c] via a dense tree over d
                    # (d-major layout: the top/bottom d-halves are contiguous
                    # 512-elem runs). Levels 1-2 stay bf16 (2x DVE mode),
                    # levels 3-5 accumulate in f32 for precision.
                    hE = E // 2  # 512
                    el1 = enbf[:, : cs * E].rearrange(
                        "p (t e) -> p t e", t=cs
                    )
                    nc.vector.tensor_tensor(
                        el1[:, :, :hE], el1[:, :, :hE], el1[:, :, hE:], Alu.add
                    )
                    nc.vector.tensor_tensor(
                        el1[:, :, : hE // 2],
                        el1[:, :, : hE // 2],
                        el1[:, :, hE // 2 : hE],
                        Alu.add,
                    )
                    qE = E // 4  # 256
                    sc3_ap = sc3[:, : cs * (E // 8)].rearrange(
                        "p (t e) -> p t e", t=cs
                    )
                    nc.vector.tensor_tensor(
                        sc3_ap,
                        el1[:, :, : qE // 2],
                        el1[:, :, qE // 2 : qE],
                        Alu.add,
                    )
                    nc.vector.tensor_tensor(
                        sc3_ap[:, :, : E // 16],
                        sc3_ap[:, :, : E // 16],
                        sc3_ap[:, :, E // 16 : E // 8],
                        Alu.add,
                    )
                    sc_ap = sc[:, : cs * K].rearrange("p (t c) -> p t c", t=cs)
                    nc.vector.tensor_tensor(
                        sc_ap,
                        sc3_ap[:, :, :K],
                        sc3_ap[:, :, K : 2 * K],
                        Alu.add,
                    )

                    # softmax over the K axis (no max subtraction needed:
                    # |scores| <= 32 so exp stays in fp32 range)
                    nc.scalar.activation(
                        ex[:, : cs * K], sc[:, : cs * K], Act.Exp
                    )
                    nc.vector.tensor_reduce(
                        rs[:, :cs],
                        ex[:, : cs * K].rearrange("p (t c) -> p t c", t=cs),
                        axis=Axis.X,
                        op=Alu.add,
                    )
                    nc.vector.reciprocal(ri[:, :cs], rs[:, :cs])
                    ri_b = ri[:, :cs].unsqueeze(2).broadcast_to([P, cs, K])
                    at_ap = at[:, : cs * K].rearrange("p (t c) -> p t c", t=cs)
                    nc.vector.tensor_tensor(
                        at_ap,
                        ex[:, : cs * K].rearrange("p (t c) -> p t c", t=cs),
                        ri_b,
                        Alu.mult,
                    )

                    # attn expanded to a dense bf16 replica (at_rep[s,d] =
                    # attn[s] for s = t*K+c) on ACT.
                    at_b = at[:, : cs * K].unsqueeze(2).broadcast_to(
                        [P, cs * K, D]
                    )
                    nc.scalar.activation(
                        at_rep[:, : cs * E].rearrange("p (s d) -> p s d", d=D),
                        at_b,
                        Act.Copy,
                    )

                    # w[t,c,d] = v[t,c,d] * attn[t,c]: dense bf16 * bf16 ->
                    # bf16 (DVE 2x mode), written into the enbf buffer.
                    nc.vector.tensor_tensor(
                        enbf[:, : cs * E], v_t, at_rep[:, : cs * E], Alu.mult
                    )

                    # out[t,d] = sum_c w[t,c,d] via a dense tree over c
                    # (c-major layout). Levels 1-4 bf16 (2x), level 5 f32.
                    wl = enbf[:, : cs * E].rearrange("p (t e) -> p t e", t=cs)
                    nc.vector.tensor_tensor(
                        wl[:, :, :hE], wl[:, :, :hE], wl[:, :, hE:], Alu.add
                    )
                    nc.vector.tensor_tensor(
                        wl[:, :, : hE // 2],
                        wl[:, :, : hE // 2],
                        wl[:, :, hE // 2 : hE],
                        Alu.add,
                    )
                    nc.vector.tensor_tensor(
                        wl[:, :, : qE // 2],
                        wl[:, :, : qE // 2],
                        wl[:, :, qE // 2 : qE],
                        Alu.add,
                    )
                    nc.vector.tensor_tensor(
                        wl[:, :, : E // 16],
                        wl[:, :, : E // 16],
                        wl[:, :, E // 16 : E // 8],
                        Alu.add,
                    )
                    on_ap = on[:, : cs * D].rearrange("p (t d) -> p t d", t=cs)
                    nc.vector.tensor_tensor(
                        on_ap,
                        wl[:, :, :D],
                        wl[:, :, D : 2 * D],
                        Alu.add,
                    )

                    nc.sync.dma_start(
                        out=out_s[:, su : su + cs],
                        in_=on[:, : cs * D].rearrange("p (s d) -> p s d", s=cs),
                    )
                su0 += nsub
            if len(SEGMENTS) <= 3 and NT * SUB > QSPLIT:
                nc.sync.dma_start(
                    out=q_full[:, QSPLIT * D :].rearrange(
                        "p (s d) -> p s d", d=D
                    ),
                    in_=qs[:].rearrange("(s p) d -> p s d", p=P)[:, QSPLIT:],
                )

    return nc


def _get_nc(general_padd: bool):
    key = bool(general_padd)
    if key not in _cache:
        nc = _build(general_padd)
        # Run the Bacc compile pipeline (register allocation, sync-wait
        # splitting, ACT table loads) before handing the module to the
        # PJRT execution path, which serializes nc.m as-is.
        nc.finalize()
        _cache[key] = nc
    return _cache[key]


def _shard(q, k, v, p_add):
    """Returns in_maps for the 8 cores. Core c gets flattened-(B*H) groups
    [2c, 2c+1]."""
    qf = np.ascontiguousarray(q, dtype=np.float32).reshape(B * H, N, D)
    kf = np.ascontiguousarray(k, dtype=np.float32).reshape(B * H, N, E)
    vf = np.ascontiguousarray(v, dtype=np.float32).reshape(B * H, N, E)
    gpc = B * H // N_CORES  # bh-groups per core (2)
    general = not np.allclose(np.asarray(p_add, dtype=np.float32), 1.0)
    in_maps = []
    for c in range(N_CORES):
        m = {
            "qs": np.ascontiguousarray(
                qf[c * gpc : (c + 1) * gpc].reshape(PTS_PER_CORE, D)
            ),
            "ks": np.ascontiguousarray(
                kf[c * gpc : (c + 1) * gpc].reshape(PTS_PER_CORE, E)
            ),
            "vs": np.ascontiguousarray(
                vf[c * gpc : (c + 1) * gpc].reshape(PTS_PER_CORE, E)
            ),
        }
        if general:
            m["pexp"] = np.ascontiguousarray(
                np.tile(
                    np.asarray(p_add, dtype=np.float32).reshape(1, D), (P, 1)
                )
            )
        in_maps.append(m)
    return in_maps, general


def _run(q, k, v, p_add, trace=False, tmpdir=None):
    from concourse.bass_utils import run_bass_kernel_spmd

    in_maps, general = _shard(q, k, v, p_add)
    nc = _get_nc(general)
    res = run_bass_kernel_spmd(
        nc, in_maps, list(range(N_CORES)), trace=trace, tmpdir=tmpdir
    )
    gpc = B * H // N_CORES
    out_full = np.empty((B, N, H, D), dtype=np.float32)
    for c in range(N_CORES):
        o = res.results[c]["out"].reshape(gpc, N, D)
        for j in range(gpc):
            bh = c * gpc + j
            out_full[bh // H, :, bh % H, :] = o[j]
    return out_full, res


def kernel(q, k, v, p_add):
    out, _ = _run(q, k, v, p_add)
    return out
